# revision 3
# baseline (speedup 1.0000x reference)
"""Trainium2 Bass kernel for nn_Attention_9594956939856 (v2).

Single-head spatial self-attention over 64x64 feature maps:
    q = Wq@x + bq, k = Wk@x + bk, v = Wv@x + bv   (1x1 convs)
    out = gamma * softmax(q^T k) @ v + x

Sharding: data-parallel over batch - 8 samples onto 8 NeuronCores; no
collectives. Per core: C=256 channels, N=4096 tokens, dk=32.

v2 design (vs v1 baseline at ~250us):
  - ALL matmuls in fp8e4m3 with DoubleRow (0.5 cyc/out-col): scores use a
    stride-0 broadcast on the o-axis of both lhsT (k j-tile) and rhs (q),
    so the K=32 contraction runs as 256 rows = 8 replicas; the 8x factor
    plus the Schraudolph scale A=8/ln2 are folded into Wq host-side, so
    the scores PSUM holds A*s directly.
  - exp split across two engines: ACT computes true Exp(scale=1/A); DVE
    computes a Schraudolph-style exp: u8 = round(min(A*s + 56 + delta,
    126)) bitcast to fp8e4m3 == e^s with ~4% error (same order as fp8
    quantization of e). delta=-0.37 calibrated offline. Negative scores
    saturate the u8 conversion at 0 == +0.0 in fp8. 126 cap avoids the
    0x7F NaN encoding.
  - denominator d = sum_j e via an all-ones DR matmul with M=128, giving
    d broadcast to all 128 PSUM partitions (no separate broadcast step).
  - v is projected directly into the transposed pair layout the
    attention-sum needs, with the PSUM written in vt8 column order so the
    cast is a contiguous 1024-col copy.
  - gamma*bv residual correction folded into x host-side (exact, fp32);
    Wv scaled by 32 host-side to keep fp8 weights in normal range; the
    32 is divided back out with gamma/32 in the finalize.
  - finalize: dr=recip(d) [DVE], dr*=gamma/32 [Pool], m=po*dr [DVE],
    t=m+x [Pool], DMA out. GPSIMD cannot touch PSUM so it only gets
    SBUF-side ops (x8 casts, final adds, gamma mul).
"""

import numpy as np

import concourse.bass as bass
import concourse.mybir as mybir
from concourse.tile import TileContext
from concourse.bass_utils import run_bass_kernel_spmd

B, C, H, W = 8, 256, 64, 64
N = H * W          # 4096 tokens
DK = C // 8        # 32
P = 128
F32 = mybir.dt.float32
BF16 = mybir.dt.bfloat16
FP8 = mybir.dt.float8e4
U8 = mybir.dt.uint8
DR = mybir.MatmulPerfMode.DoubleRow
AF = mybir.ActivationFunctionType
ALU = mybir.AluOpType

NJT = N // P       # 32 j-tiles
NJP = NJT // 2     # 16 j-pairs
HCH = 512          # i-chunk width
NCH = N // HCH     # 8 i-chunks

EXP_A = 8.0 / np.log(2.0)          # 11.5416 (fp8e4m3 log2 scale)
EXP_BIAS = 56.0 - 0.37             # e4m3 exponent bias*8 + calibrated delta

# exp engine per j-pair: 'A' = ACT true exp, 'V' = DVE u8 Schraudolph
VARIANT = {
    "exp_pat": "AVAVAVAVAVAVAVAA",  # 9 ACT / 7 DVE
    "vt_cast_v": True,              # vt8 casts on DVE (else ACT)
}


# ---------------------------------------------------------------------------
# Workaround: the walrus build in this container allows only ONE sync wait
# per instruction ("Too many sync wait commands"), but Tile's wait
# assignment attaches up to 2 (and the tail drain more). Hoist all-but-one
# wait of any over-subscribed instruction onto dedicated same-engine nofuse
# nops inserted immediately before it in the ordered stream.
_PATCHED = False


def _apply_tile_patch():
    global _PATCHED
    if _PATCHED:
        return
    from concourse.tile import TileContext as TC
    from concourse.vector_clock import ScopedClock, VectorClock

    def _drain_and_barrier_split(self, tick_clock, wait_clock):
        gc = tick_clock.global_clock
        n = len(gc)
        for i in range(n):
            if gc[i] > 0:
                vec = [0] * n
                vec[i] = gc[i]
                ins = self.nc.sync.nop(nofuse=True, hint="tail_drain_wait")
                wait_clock.add_sem_waits(
                    ins.ins, ScopedClock({None: VectorClock(vec)})
                )
        self.nc.sync.drain()
        self.nc.all_engine_barrier()
        assert self.sems is not None
        popped = self.nc._tile_sem_poison_stack.pop()
        assert popped is self._sem_poison
        self.nc.clear_and_free_semaphores(list(self.sems.allocated().values()))
        self.nc.all_engine_barrier()

    TC._drain_and_barrier = _drain_and_barrier_split

    orig_lower = TC._lower_ordered_insts
    counter = [0]

    def _lower_split_waits(self, ordered):
        for bb_name, insts in ordered.items():
            new = []
            changed = False
            for inst in insts:
                si = inst.sync_info
                if si is not None and len(si.on_wait) > 1:
                    changed = True
                    waits = list(si.on_wait)
                    for w in waits[:-1]:
                        counter[0] += 1
                        new.append(
                            mybir.InstNoOp(
                                name=f"splitw-{counter[0]}",
                                sync_info=mybir.SyncInfo(
                                    on_wait=[w], on_update=[]
                                ),
                                bass_nofuse=True,
                                engine=inst.engine,
                            )
                        )
                    inst.sync_info = mybir.SyncInfo(
                        on_wait=[waits[-1]], on_update=list(si.on_update)
                    )
                new.append(inst)
            if changed:
                insts[:] = new
        return orig_lower(self, ordered)

    TC._lower_ordered_insts = _lower_split_waits
    _PATCHED = True


def _bcast2(ap):
    """(128, M) -> (128, 2, M) with a stride-0 middle axis (DR o-pairs)."""
    m = ap.shape[-1]
    return ap.unsqueeze(1).broadcast_to((P, 2, m))


def _emit_body(nc, tc, pools, ext):
    consts, big, epool, fin, ps_s_pool, ps_acc_pool = pools
    x_e, wq8_e, wk8_e, wv8_e, bq_e, bk_e, gam_e, y_e = ext

    # ---- constants / weights ---------------------------------------------
    wq8 = consts.tile([P, 2 * P], FP8, tag="wq8")
    wk8 = consts.tile([P, 2 * P], FP8, tag="wk8")
    wv8 = consts.tile([P, 2 * C], FP8, tag="wv8")
    bq_t = consts.tile([P, 1], F32, tag="bq_t")
    bk_t = consts.tile([P, 1], F32, tag="bk_t")
    gam_t = consts.tile([P, 1], F32, tag="gam_t")
    ones8 = consts.tile([P, 2 * P], FP8, tag="ones8")

    nc.sync.dma_start(out=wq8[:], in_=wq8_e[:])
    nc.sync.dma_start(out=wk8[:], in_=wk8_e[:])
    nc.sync.dma_start(out=wv8[:], in_=wv8_e[:])
    nc.sync.dma_start(out=bq_t[:], in_=bq_e[:])
    nc.sync.dma_start(out=bk_t[:], in_=bk_e[:])
    nc.sync.dma_start(out=gam_t[:], in_=gam_e[:])
    nc.vector.memset(ones8[:], 1.0)

    wq_ap = wq8[:].rearrange("p (o m) -> p o m", o=2)
    wk_ap = wk8[:].rearrange("p (o m) -> p o m", o=2)
    wv_ap = wv8[:].rearrange("p (o e) -> p o e", o=2)
    ones_ap = ones8[:].rearrange("p (o m) -> p o m", o=2)

    # ---- big SBUF tensors -------------------------------------------------
    xf0 = big.tile([P, N], F32, tag="xf0")     # channels 0..127 (+g*bv)
    xf1 = big.tile([P, N], F32, tag="xf1")     # channels 128..255
    x8 = big.tile([P, 2 * N], FP8, tag="x8")   # pair layout [o*N + n]
    q8 = big.tile([P, N], FP8, tag="q8")       # 4 replicas of 32 q-dims
    k8 = big.tile([P, N], FP8, tag="k8")
    vt8 = big.tile([P, NJP * 512], FP8, tag="vt8")

    x8_ap = x8[:].rearrange("p (o n) -> p o n", o=2)

    # ---- prologue: x load, fp8 cast, k/v/q projections --------------------
    for nch in range(NCH):
        sl = slice(nch * HCH, (nch + 1) * HCH)
        nc.sync.dma_start(out=xf0[:, sl], in_=x_e[0:P, sl])
        nc.sync.dma_start(out=xf1[:, sl], in_=x_e[P : 2 * P, sl])
        nc.gpsimd.tensor_copy(x8_ap[:, 0, sl], xf0[:, sl])
        nc.gpsimd.tensor_copy(x8_ap[:, 1, sl], xf1[:, sl])

    # k projection: 2 chunks per PSUM tile, one biased-cast per 1024 cols
    for half in range(NCH // 2):
        pk = ps_s_pool.tile([P, 2 * HCH], F32, tag="ps", bufs=2)
        for o in range(2):
            nch = 2 * half + o
            sl = slice(nch * HCH, (nch + 1) * HCH)
            nc.tensor.matmul(
                pk[:, o * HCH : (o + 1) * HCH], wk_ap, x8_ap[:, :, sl],
                start=True, stop=True, perf_mode=DR,
            )
        osl = slice(half * 2 * HCH, (half + 1) * 2 * HCH)
        nc.scalar.activation(k8[:, osl], pk[:], AF.Identity, bias=bk_t[:])

    # v projection: 4 j-tiles (2 pairs) per PSUM tile, written in vt8
    # column order (pr*512 + h*256 + o*128 + m), then one contiguous
    # 1024-col cast
    for grp in range(NJT // 4):
        pv = ps_s_pool.tile([P, 2 * HCH], F32, tag="ps", bufs=2)
        pv_hom = pv[:].rearrange("p (r h o m) -> p r h o m", r=2, h=2, o=2)
        for pr in range(2):
            for o in range(2):
                jt = 4 * grp + 2 * pr + o
                nsl = slice(jt * P, (jt + 1) * P)
                nc.tensor.matmul(
                    pv_hom[:, pr, :, o, :], x8_ap[:, :, nsl], wv_ap,
                    start=True, stop=True, perf_mode=DR,
                )
        vsl = slice(grp * 1024, (grp + 1) * 1024)
        if VARIANT["vt_cast_v"]:
            nc.vector.tensor_copy(vt8[:, vsl], pv[:])
        else:
            nc.scalar.activation(vt8[:, vsl], pv[:], AF.Copy)

    # q projection (after k/v so chunk-0 scores can start asap)
    for half in range(NCH // 2):
        pq = ps_s_pool.tile([P, 2 * HCH], F32, tag="ps", bufs=2)
        for o in range(2):
            nch = 2 * half + o
            sl = slice(nch * HCH, (nch + 1) * HCH)
            nc.tensor.matmul(
                pq[:, o * HCH : (o + 1) * HCH], wq_ap, x8_ap[:, :, sl],
                start=True, stop=True, perf_mode=DR,
            )
        osl = slice(half * 2 * HCH, (half + 1) * 2 * HCH)
        nc.scalar.activation(q8[:, osl], pq[:], AF.Identity, bias=bq_t[:])

    # ---- main attention loop over i-chunks -------------------------------
    pat = VARIANT["exp_pat"]
    for ich in range(NCH):
        isl = slice(ich * HCH, (ich + 1) * HCH)
        q_b = _bcast2(q8[:, isl])
        es = []
        for jp in range(NJP):
            ps = ps_s_pool.tile([P, 2 * HCH], F32, tag="ps", bufs=2)
            for o in range(2):
                jt = 2 * jp + o
                k_b = _bcast2(k8[:, jt * P : (jt + 1) * P])
                nc.tensor.matmul(
                    ps[:, o * HCH : (o + 1) * HCH], k_b, q_b,
                    start=True, stop=True, perf_mode=DR,
                )
            e8 = epool.tile([P, 2 * HCH], FP8, tag="e", bufs=24)
            if pat[jp] == "A":
                nc.scalar.activation(
                    e8[:], ps[:], AF.Exp, scale=float(1.0 / EXP_A)
                )
            else:
                nc.vector.tensor_scalar(
                    e8[:].bitcast(U8), ps[:], EXP_BIAS, 126.0,
                    op0=ALU.add, op1=ALU.min,
                )
            es.append(e8)

        po0 = ps_acc_pool.tile([P, HCH], F32, tag="po", bufs=2)
        po1 = ps_acc_pool.tile([P, HCH], F32, tag="po", bufs=2)
        pd = ps_acc_pool.tile([P, HCH], F32, tag="pd", bufs=2)
        for jp in range(NJP):
            rhs = es[jp][:].rearrange("p (o i) -> p o i", o=2)
            st, sp = jp == 0, jp == NJP - 1
            for h, po in ((0, po0), (1, po1)):
                lhsT = vt8[
                    :, jp * 512 + h * 2 * P : jp * 512 + (h + 1) * 2 * P
                ].rearrange("p (o m) -> p o m", o=2)
                nc.tensor.matmul(
                    po[:], lhsT, rhs, start=st, stop=sp, perf_mode=DR
                )
            nc.tensor.matmul(
                pd[:], ones_ap, rhs, start=st, stop=sp, perf_mode=DR
            )

        # finalize: out = po * (gamma/32 / d) + x
        dr = fin.tile([P, HCH], F32, tag="dr", bufs=2)
        nc.vector.reciprocal(dr[:], pd[:])
        nc.gpsimd.tensor_scalar_mul(dr[:], dr[:], gam_t[:])
        m0 = fin.tile([P, HCH], F32, tag="m0", bufs=2)
        t0 = fin.tile([P, HCH], F32, tag="t0", bufs=2)
        nc.vector.tensor_tensor(m0[:], po0[:], dr[:], op=ALU.mult)
        nc.gpsimd.tensor_tensor(t0[:], m0[:], xf0[:, isl], op=ALU.add)
        nc.sync.dma_start(out=y_e[0:P, isl], in_=t0[:])
        m1 = fin.tile([P, HCH], F32, tag="m1", bufs=2)
        t1 = fin.tile([P, HCH], F32, tag="t1", bufs=2)
        nc.vector.tensor_tensor(m1[:], po1[:], dr[:], op=ALU.mult)
        nc.gpsimd.tensor_tensor(t1[:], m1[:], xf1[:, isl], op=ALU.add)
        nc.sync.dma_start(out=y_e[P : 2 * P, isl], in_=t1[:])


def build_bass(loop_n: int | None = None) -> bass.Bass:
    """Build the kernel. loop_n wraps the body in a device-side For_i loop
    (with a tiny 'tick' sentinel output) for slope-based benchmarking."""
    _apply_tile_patch()
    nc = bass.Bass()

    x_e = nc.declare_dram_parameter("x", [C, N], F32, isOutput=False)
    wq8_e = nc.declare_dram_parameter("wq8", [P, 2 * P], FP8, isOutput=False)
    wk8_e = nc.declare_dram_parameter("wk8", [P, 2 * P], FP8, isOutput=False)
    wv8_e = nc.declare_dram_parameter("wv8", [P, 2 * C], FP8, isOutput=False)
    bq_e = nc.declare_dram_parameter("bq_r", [P, 1], F32, isOutput=False)
    bk_e = nc.declare_dram_parameter("bk_r", [P, 1], F32, isOutput=False)
    gam_e = nc.declare_dram_parameter("gam_b", [P, 1], F32, isOutput=False)
    y_e = nc.declare_dram_parameter("y", [C, N], F32, isOutput=True)
    tick_e = None
    if loop_n is not None:
        tick_e = nc.declare_dram_parameter("tick", [1, 8], F32, isOutput=True)

    ext = (x_e, wq8_e, wk8_e, wv8_e, bq_e, bk_e, gam_e, y_e)

    with (
        TileContext(nc) as tc,
        tc.tile_pool(name="consts", bufs=1) as consts,
        tc.tile_pool(name="big", bufs=2) as big,
        tc.tile_pool(name="epool", bufs=24) as epool,
        tc.tile_pool(name="fin", bufs=2) as fin,
        tc.tile_pool(name="ps_s", bufs=2, space="PSUM") as ps_s_pool,
        tc.tile_pool(name="ps_acc", bufs=2, space="PSUM") as ps_acc_pool,
    ):
        pools = (consts, big, epool, fin, ps_s_pool, ps_acc_pool)
        if loop_n is None:
            _emit_body(nc, tc, pools, ext)
        else:
            with tc.For_i(0, loop_n, 1):
                _emit_body(nc, tc, pools, ext)
            t = fin.tile([1, 8], F32, tag="tick")
            nc.vector.memset(t[:], 1.0)
            nc.sync.dma_start(out=tick_e[:], in_=t[:])

    return nc


_NC_CACHE = None


def _get_nc() -> bass.Bass:
    global _NC_CACHE
    if _NC_CACHE is None:
        _NC_CACHE = build_bass()
    return _NC_CACHE


def prep_core_inputs(x, Wq, bq, Wk, bk, Wv, bv, gamma):
    f8 = mybir.dt.np(FP8)
    x = np.asarray(x, np.float32).reshape(B, C, N)
    Wq = np.asarray(Wq, np.float32)
    Wk = np.asarray(Wk, np.float32)
    Wv = np.asarray(Wv, np.float32)
    bq = np.asarray(bq, np.float32)
    bk = np.asarray(bk, np.float32)
    bv = np.asarray(bv, np.float32)
    g = float(np.asarray(gamma, np.float32).reshape(-1)[0])

    # residual absorbs gamma*bv (exact): out = gamma*(v_hat@attn) + (x+g*bv)
    xadj = x + (g * bv)[None, :, None]

    rep = np.tile(np.arange(DK), P // DK)  # m -> m%32
    # wq carries A/8: 8x stride-0-DR overcount and the Schraudolph scale,
    # so the scores PSUM holds A*s directly
    sq = EXP_A / 8.0
    wq8 = np.concatenate(
        [(sq * Wq.T[0:P])[:, rep], (sq * Wq.T[P : 2 * P])[:, rep]], axis=1
    ).astype(f8)
    wk8 = np.concatenate(
        [Wk.T[0:P][:, rep], Wk.T[P : 2 * P][:, rep]], axis=1
    ).astype(f8)
    # Wv scaled by 32 to keep fp8 weights in normal range (undone by gam_b)
    wv8 = np.concatenate([32.0 * Wv.T[0:P], 32.0 * Wv.T[P : 2 * P]], axis=1
                         ).astype(f8)
    bq_r = np.ascontiguousarray((sq * bq)[rep]).reshape(P, 1)
    bk_r = np.ascontiguousarray(bk[rep]).reshape(P, 1)
    gam_b = np.full((P, 1), g / 32.0, np.float32)

    shared = {
        "wq8": wq8, "wk8": wk8, "wv8": wv8,
        "bq_r": bq_r, "bk_r": bk_r, "gam_b": gam_b,
    }
    return [
        {"x": np.ascontiguousarray(xadj[b]), **shared} for b in range(B)
    ]


def kernel(**inputs) -> np.ndarray:
    nc = _get_nc()
    in_maps = prep_core_inputs(**inputs)
    res = run_bass_kernel_spmd(nc, in_maps, list(range(B)))
    y = np.stack([res.results[i]["y"] for i in range(B)])
    return np.ascontiguousarray(y.reshape(B, C, H, W).astype(np.float32))


# revision 6
# speedup vs baseline: 1.1533x; 1.1533x over previous
"""Trainium2 Bass kernel for nn_Attention_9594956939856 (v3).

Single-head spatial self-attention over 64x64 feature maps:
    q = Wq@x + bq, k = Wk@x + bk, v = Wv@x + bv   (1x1 convs)
    out = gamma * softmax(q^T k) @ v + x

Sharding: data-parallel over batch - 8 samples onto 8 NeuronCores; no
collectives. Per core: C=256 channels, N=4096 tokens, dk=32.

Design (engine costs MEASURED on this hardware via micro.py, which
disagrees badly with the shipped cost model):
  - bf16 matmuls stream 2 cols/cycle here (~109ns per (128,512) out), fp8
    DoubleRow ~142ns. Scores use v1's bf16 quadrant scheme (k j-tile
    stationary at PE row 64*o, K=32 of a 4x-replicated q/k) computing the
    TRANSPOSED scores s'[j,i] so the attention-weighted sum needs no
    transpose of the huge matrix.
  - ACT exp is very fast here (227ns per (128,1024) PSUM->fp8), so ALL
    exp runs on ACT. ACT Copy/Identity are SLOW (0.8-1.6us) - ACT does
    exp ONLY.
  - accumulation: fp8 DoubleRow over j-pairs: po0/po1 (2x128 channels,
    vT stationary) and the denominator pd via an all-ones lhsT with
    M=128, which lands d[i] broadcast across all 128 PSUM partitions
    (no quadrant-sum / broadcast matmul needed).
  - the accumulation matmuls are software-pipelined LAG pairs behind the
    scores matmuls, so the in-order PE never sits in a pure-scores phase
    stalled on exp slot-reuse (the v1 structure serialized here).
  - PSUM->SBUF casts (q/k bias-add, vt8) on DVE; final residual adds on
    GPSIMD (SBUF-only engine); gamma*bv residual term folded into x
    host-side (exact).
"""

import numpy as np
import ml_dtypes

import concourse.bass as bass
import concourse.mybir as mybir
from concourse.tile import TileContext
from concourse.bass_utils import run_bass_kernel_spmd

B, C, H, W = 8, 256, 64, 64
N = H * W          # 4096 tokens
DK = C // 8        # 32
P = 128
F32 = mybir.dt.float32
BF16 = mybir.dt.bfloat16
FP8 = mybir.dt.float8e4
DR = mybir.MatmulPerfMode.DoubleRow
AF = mybir.ActivationFunctionType
ALU = mybir.AluOpType

NJT = N // P       # 32 j-tiles
NJP = NJT // 2     # 16 j-pairs
HCH = 512          # i-chunk width
NCH = N // HCH     # 8 i-chunks

VARIANT = {
    "acc_lag": 3,        # j-pairs of lookahead before accumulation
    "xb_on_pool": False,  # xb casts on gpsimd instead of DVE
}


# ---------------------------------------------------------------------------
# Workaround: the walrus build in this container allows only ONE sync wait
# per instruction ("Too many sync wait commands"), but Tile's wait
# assignment attaches up to 2 (and the tail drain more). Hoist all-but-one
# wait of any over-subscribed instruction onto dedicated same-engine nofuse
# nops inserted immediately before it in the ordered stream.
_PATCHED = False


def _apply_tile_patch():
    global _PATCHED
    if _PATCHED:
        return
    from concourse.tile import TileContext as TC
    from concourse.vector_clock import ScopedClock, VectorClock

    def _drain_and_barrier_split(self, tick_clock, wait_clock):
        gc = tick_clock.global_clock
        n = len(gc)
        for i in range(n):
            if gc[i] > 0:
                vec = [0] * n
                vec[i] = gc[i]
                ins = self.nc.sync.nop(nofuse=True, hint="tail_drain_wait")
                wait_clock.add_sem_waits(
                    ins.ins, ScopedClock({None: VectorClock(vec)})
                )
        self.nc.sync.drain()
        self.nc.all_engine_barrier()
        assert self.sems is not None
        popped = self.nc._tile_sem_poison_stack.pop()
        assert popped is self._sem_poison
        self.nc.clear_and_free_semaphores(list(self.sems.allocated().values()))
        self.nc.all_engine_barrier()

    TC._drain_and_barrier = _drain_and_barrier_split

    orig_lower = TC._lower_ordered_insts
    counter = [0]

    def _lower_split_waits(self, ordered):
        for bb_name, insts in ordered.items():
            new = []
            changed = False
            for inst in insts:
                si = inst.sync_info
                if si is not None and len(si.on_wait) > 1:
                    changed = True
                    waits = list(si.on_wait)
                    for w in waits[:-1]:
                        counter[0] += 1
                        new.append(
                            mybir.InstNoOp(
                                name=f"splitw-{counter[0]}",
                                sync_info=mybir.SyncInfo(
                                    on_wait=[w], on_update=[]
                                ),
                                bass_nofuse=True,
                                engine=inst.engine,
                            )
                        )
                    inst.sync_info = mybir.SyncInfo(
                        on_wait=[waits[-1]], on_update=list(si.on_update)
                    )
                new.append(inst)
            if changed:
                insts[:] = new
        return orig_lower(self, ordered)

    TC._lower_ordered_insts = _lower_split_waits
    _PATCHED = True


def _emit_body(nc, tc, pools, ext):
    consts, big, epool, fin, ps_s_pool, ps_acc_pool = pools
    x_e, wqt_e, wkt_e, wvt_e, bq_e, bk_e, gam_e, y_e = ext

    # ---- constants / weights ---------------------------------------------
    wqt_a = consts.tile([P, P], BF16, tag="wqt_a")
    wqt_b = consts.tile([P, P], BF16, tag="wqt_b")
    wkt_a = consts.tile([P, P], BF16, tag="wkt_a")
    wkt_b = consts.tile([P, P], BF16, tag="wkt_b")
    wvt_a = consts.tile([P, C], BF16, tag="wvt_a")
    wvt_b = consts.tile([P, C], BF16, tag="wvt_b")
    bq_t = consts.tile([P, 1], F32, tag="bq_t")
    bk_t = consts.tile([P, 1], F32, tag="bk_t")
    gam_t = consts.tile([P, 1], F32, tag="gam_t")
    ones8 = consts.tile([P, 2 * P], FP8, tag="ones8")

    nc.sync.dma_start(out=wqt_a[:], in_=wqt_e[0:P, :])
    nc.sync.dma_start(out=wqt_b[:], in_=wqt_e[P : 2 * P, :])
    nc.sync.dma_start(out=wkt_a[:], in_=wkt_e[0:P, :])
    nc.sync.dma_start(out=wkt_b[:], in_=wkt_e[P : 2 * P, :])
    nc.sync.dma_start(out=wvt_a[:], in_=wvt_e[0:P, :])
    nc.sync.dma_start(out=wvt_b[:], in_=wvt_e[P : 2 * P, :])
    nc.sync.dma_start(out=bq_t[:], in_=bq_e[:])
    nc.sync.dma_start(out=bk_t[:], in_=bk_e[:])
    nc.sync.dma_start(out=gam_t[:], in_=gam_e[:])
    nc.vector.memset(ones8[:], 1.0)
    ones_ap = ones8[:].rearrange("p (o m) -> p o m", o=2)

    # ---- big SBUF tensors -------------------------------------------------
    xf0 = big.tile([P, N], F32, tag="xf0")     # channels 0..127 (+g*bv)
    xf1 = big.tile([P, N], F32, tag="xf1")
    xb0 = big.tile([P, N], BF16, tag="xb0")
    xb1 = big.tile([P, N], BF16, tag="xb1")
    q_rep = big.tile([P, N], BF16, tag="q_rep")
    k_rep = big.tile([P, N], BF16, tag="k_rep")
    # vt8 pair layout: block jp holds cols jp*512 + h*256 + o*128 + m ==
    # vT[j=(2jp+o)*128+p, channel h*128+m]
    vt8 = big.tile([P, NJP * 512], FP8, tag="vt8")

    # ---- prologue: x load + bf16 casts + projections ---------------------
    cast_eng = nc.gpsimd if VARIANT["xb_on_pool"] else nc.vector
    for nch in range(NCH):
        sl = slice(nch * HCH, (nch + 1) * HCH)
        nc.sync.dma_start(out=xf0[:, sl], in_=x_e[0:P, sl])
        nc.sync.dma_start(out=xf1[:, sl], in_=x_e[P : 2 * P, sl])
        cast_eng.tensor_copy(xb0[:, sl], xf0[:, sl])
        cast_eng.tensor_copy(xb1[:, sl], xf1[:, sl])

    # k projection: 1024-col PSUM tiles (2 i-chunks), one biased cast each
    for proj_w, proj_bias, proj_out in (
        (("wk", wkt_a, wkt_b), bk_t, k_rep),
        (("wq", wqt_a, wqt_b), bq_t, q_rep),
    ):
        _, w_a, w_b = proj_w
        for half in range(NCH // 2):
            pk = ps_s_pool.tile([P, 2 * HCH], F32, tag="ps", bufs=2)
            for o in range(2):
                sl = slice((2 * half + o) * HCH, (2 * half + o + 1) * HCH)
                psl = slice(o * HCH, (o + 1) * HCH)
                nc.tensor.matmul(
                    pk[:, psl], w_a, xb0[:, sl], start=True, stop=False
                )
                nc.tensor.matmul(
                    pk[:, psl], w_b, xb1[:, sl], start=False, stop=True
                )
            osl = slice(half * 2 * HCH, (half + 1) * 2 * HCH)
            nc.vector.tensor_scalar_add(proj_out[:, osl], pk[:], proj_bias[:])

    # v projection: 4 j-tiles per PSUM tile at natural (jloc, h, m) order,
    # one rearranging fp8 cast into the vt8 pair layout per group
    for grp in range(NJT // 4):
        pv = ps_s_pool.tile([P, 2 * HCH], F32, tag="ps", bufs=2)
        for jloc in range(4):
            jt = 4 * grp + jloc
            nsl = slice(jt * P, (jt + 1) * P)
            psl = slice(jloc * C, (jloc + 1) * C)
            nc.tensor.matmul(
                pv[:, psl], xb0[:, nsl], wvt_a, start=True, stop=False
            )
            nc.tensor.matmul(
                pv[:, psl], xb1[:, nsl], wvt_b, start=False, stop=True
            )
        # read (pr, o, h, m) [strides 512,256,128,1]; write vt8 (pr, h, o, m)
        src = pv[:].rearrange("p (r o h m) -> p r o h m", r=2, o=2, h=2)
        dst = vt8[
            :, grp * 1024 : (grp + 1) * 1024
        ].rearrange("p (r h o m) -> p r o h m", r=2, h=2, o=2)
        nc.vector.tensor_copy(dst, src)

    # ---- main attention loop over i-chunks -------------------------------
    # Pair jp's scores+exp emit together with pair jp-LAG's accumulation,
    # keeping the in-order PE busy while ACT drains the exp queue.
    LAG = VARIANT["acc_lag"]
    for ich in range(NCH):
        isl = slice(ich * HCH, (ich + 1) * HCH)
        es = []

        po0 = ps_acc_pool.tile([P, HCH], F32, tag="po", bufs=2)
        po1 = ps_acc_pool.tile([P, HCH], F32, tag="po", bufs=2)
        pd = ps_acc_pool.tile([P, HCH], F32, tag="pd", bufs=2)

        def _accum(jp):
            rhs = es[jp][:].rearrange("p (o i) -> p o i", o=2)
            st, sp = jp == 0, jp == NJP - 1
            nc.tensor.matmul(
                pd[:], ones_ap, rhs, start=st, stop=sp, perf_mode=DR
            )
            for h, po in ((0, po0), (1, po1)):
                lhsT = vt8[
                    :, jp * 512 + h * 2 * P : jp * 512 + (h + 1) * 2 * P
                ].rearrange("p (o m) -> p o m", o=2)
                nc.tensor.matmul(
                    po[:], lhsT, rhs, start=st, stop=sp, perf_mode=DR
                )

        for jp in range(NJP + LAG):
            if jp < NJP:
                ps = ps_s_pool.tile([P, 2 * HCH], F32, tag="ps", bufs=2)
                for o in range(2):
                    jt = 2 * jp + o
                    nc.tensor.matmul(
                        ps[:, o * HCH : (o + 1) * HCH],
                        k_rep[64 * o : 64 * o + DK, jt * P : (jt + 1) * P],
                        q_rep[64 * o : 64 * o + DK, isl],
                        start=True, stop=True,
                        tile_position=(64 * o, 0),
                    )
                e8 = epool.tile([P, 2 * HCH], FP8, tag="e", bufs=24)
                nc.scalar.activation(e8[:], ps[:], AF.Exp)
                es.append(e8)
            if jp >= LAG:
                _accum(jp - LAG)

        # finalize: out = po * (gamma / d) + x
        dr = fin.tile([P, HCH], F32, tag="dr", bufs=2)
        nc.vector.reciprocal(dr[:], pd[:])
        nc.vector.tensor_scalar_mul(dr[:], dr[:], gam_t[:])
        m0 = fin.tile([P, HCH], F32, tag="m0", bufs=2)
        t0 = fin.tile([P, HCH], F32, tag="t0", bufs=2)
        nc.vector.tensor_tensor(m0[:], po0[:], dr[:], op=ALU.mult)
        nc.gpsimd.tensor_tensor(t0[:], m0[:], xf0[:, isl], op=ALU.add)
        nc.sync.dma_start(out=y_e[0:P, isl], in_=t0[:])
        m1 = fin.tile([P, HCH], F32, tag="m1", bufs=2)
        t1 = fin.tile([P, HCH], F32, tag="t1", bufs=2)
        nc.vector.tensor_tensor(m1[:], po1[:], dr[:], op=ALU.mult)
        nc.gpsimd.tensor_tensor(t1[:], m1[:], xf1[:, isl], op=ALU.add)
        nc.sync.dma_start(out=y_e[P : 2 * P, isl], in_=t1[:])


def build_bass(loop_n: int | None = None) -> bass.Bass:
    """Build the kernel. loop_n wraps the body in a device-side For_i loop
    (with a tiny 'tick' sentinel output) for slope-based benchmarking."""
    _apply_tile_patch()
    nc = bass.Bass()

    x_e = nc.declare_dram_parameter("x", [C, N], F32, isOutput=False)
    wqt_e = nc.declare_dram_parameter("wqt", [C, P], BF16, isOutput=False)
    wkt_e = nc.declare_dram_parameter("wkt", [C, P], BF16, isOutput=False)
    wvt_e = nc.declare_dram_parameter("wvt", [C, C], BF16, isOutput=False)
    bq_e = nc.declare_dram_parameter("bq_r", [P, 1], F32, isOutput=False)
    bk_e = nc.declare_dram_parameter("bk_r", [P, 1], F32, isOutput=False)
    gam_e = nc.declare_dram_parameter("gam_b", [P, 1], F32, isOutput=False)
    y_e = nc.declare_dram_parameter("y", [C, N], F32, isOutput=True)
    tick_e = None
    if loop_n is not None:
        tick_e = nc.declare_dram_parameter("tick", [1, 8], F32, isOutput=True)

    ext = (x_e, wqt_e, wkt_e, wvt_e, bq_e, bk_e, gam_e, y_e)

    with (
        TileContext(nc) as tc,
        tc.tile_pool(name="consts", bufs=1) as consts,
        tc.tile_pool(name="big", bufs=2) as big,
        tc.tile_pool(name="epool", bufs=24) as epool,
        tc.tile_pool(name="fin", bufs=2) as fin,
        tc.tile_pool(name="ps_s", bufs=2, space="PSUM") as ps_s_pool,
        tc.tile_pool(name="ps_acc", bufs=2, space="PSUM") as ps_acc_pool,
    ):
        pools = (consts, big, epool, fin, ps_s_pool, ps_acc_pool)
        if loop_n is None:
            _emit_body(nc, tc, pools, ext)
        else:
            with tc.For_i(0, loop_n, 1):
                _emit_body(nc, tc, pools, ext)
            t = fin.tile([1, 8], F32, tag="tick")
            nc.vector.memset(t[:], 1.0)
            nc.sync.dma_start(out=tick_e[:], in_=t[:])

    return nc


_NC_CACHE = None


def _get_nc() -> bass.Bass:
    global _NC_CACHE
    if _NC_CACHE is None:
        _NC_CACHE = build_bass()
    return _NC_CACHE


def prep_core_inputs(x, Wq, bq, Wk, bk, Wv, bv, gamma):
    x = np.asarray(x, np.float32).reshape(B, C, N)
    Wq = np.asarray(Wq, np.float32)
    Wk = np.asarray(Wk, np.float32)
    Wv = np.asarray(Wv, np.float32)
    bq = np.asarray(bq, np.float32)
    bk = np.asarray(bk, np.float32)
    bv = np.asarray(bv, np.float32)
    g = float(np.asarray(gamma, np.float32).reshape(-1)[0])

    # residual absorbs gamma*bv (exact): out = gamma*(v_hat@attn) + (x+g*bv)
    xadj = x + (g * bv)[None, :, None]

    wqt = np.ascontiguousarray(np.tile(Wq.T, (1, 4))).astype(
        ml_dtypes.bfloat16
    )
    wkt = np.ascontiguousarray(np.tile(Wk.T, (1, 4))).astype(
        ml_dtypes.bfloat16
    )
    wvt = np.ascontiguousarray(Wv.T).astype(ml_dtypes.bfloat16)
    bq_r = np.ascontiguousarray(np.tile(bq, 4)).reshape(P, 1)
    bk_r = np.ascontiguousarray(np.tile(bk, 4)).reshape(P, 1)
    gam_b = np.full((P, 1), g, np.float32)

    shared = {
        "wqt": wqt, "wkt": wkt, "wvt": wvt,
        "bq_r": bq_r, "bk_r": bk_r, "gam_b": gam_b,
    }
    return [
        {"x": np.ascontiguousarray(xadj[b]), **shared} for b in range(B)
    ]


def kernel(**inputs) -> np.ndarray:
    nc = _get_nc()
    in_maps = prep_core_inputs(**inputs)
    res = run_bass_kernel_spmd(nc, in_maps, list(range(B)))
    y = np.stack([res.results[i]["y"] for i in range(B)])
    return np.ascontiguousarray(y.reshape(B, C, H, W).astype(np.float32))
